# revision 21
# baseline (speedup 1.0000x reference)
"""
Trainium2 Bass kernel for nn_MultiHeadHierarchicalAttention.

Sharding: 8 cores = (batch b in 0..3) x (block-half in 0..1).
Each core handles one batch and 16 of the 32 blocks for the token-level
attention; the (small) sentence-level attention + fc1 branch is computed
redundantly on both cores of a batch, scaled by 0.5, and the host sums the
two per-batch partial outputs (the final fc is linear, so partial ctx_w
contributions simply add).

Device layouts (per core, partition dim first):
  qT   [D, LQ]      kwT/vwT [D, 4096]   ksT/vsT [D, 32] (block-rolled)
  projections keep features on partitions (kw) or tokens on partitions (vw)
  token scores are computed as [t, q] tiles (K=dk=64), exp'd on ACT into
  bf16, and consumed per block by S3 matmuls producing [q, dv+1] partials
  (the +1 "ones" column of vw gives the softmax denominator for free).
  The sentence-attention factor attn_s/denom is applied with fused DVE
  scalar_tensor_tensor accumulation. ctx_w is PE-transposed at the end and
  fused into the final fc, which is emitted as outT [D, LQ] (host transposes).
"""

import sys

sys.path.insert(0, "/opt/trn_rl_repo")

import numpy as np
import concourse.bass as bass
import concourse.tile as tile
from concourse import mybir
from concourse.bass_utils import run_bass_kernel_spmd
from concourse.vector_clock import ScopedClock
from concourse.masks import make_identity

# ---------------------------------------------------------------- constants
B, LQ, NB, NT = 4, 512, 32, 256
D, H, DK, DV = 512, 8, 64, 64
NBH = NB // 2  # blocks per core
NTOK = NBH * NT  # tokens per core = 4096
NTC = NTOK // 128  # 32 token chunks of 128
SCALE = 0.125
FP = mybir.dt.float32
FR = mybir.dt.float32r
BF = mybir.dt.bfloat16
N_CORES = 8

AX = mybir.AxisListType.X
ALU = mybir.AluOpType
ACTF = mybir.ActivationFunctionType


# --------------------------------------------------------- drain workaround
def _patched_drain_and_barrier(self, tick_clock, wait_clock):
    # walrus in this container rejects >1 sem wait on a single TPB_CTRL
    # instruction ("Too many sync wait commands"); split the kernel-tail
    # drain waits across one-wait NOPs.
    nop_inst = self.nc.sync.nop(nofuse=True)
    wait_clock.add_sem_waits(nop_inst.ins, ScopedClock({None: tick_clock.global_clock}))
    waits = list(nop_inst.ins.sync_info.on_wait or [])
    if len(waits) > 1:
        nop_inst.ins.sync_info.on_wait = waits[:1]
        rest = waits[1:]
        while rest:
            extra = self.nc.sync.nop(nofuse=True)
            if extra.ins.sync_info is None:
                extra.ins.sync_info = mybir.SyncInfo(on_wait=[], on_update=[])
            extra.ins.sync_info.on_wait = rest[:1]
            rest = rest[1:]
    self.nc.sync.drain()
    self.nc.all_engine_barrier()
    assert self.sems is not None
    popped = self.nc._tile_sem_poison_stack.pop()
    assert popped is self._sem_poison
    self.nc.clear_and_free_semaphores(list(self.sems.allocated().values()))
    self.nc.all_engine_barrier()


tile.TileContext._drain_and_barrier = _patched_drain_and_barrier


def _r(ap):
    """View an f32 AP as float32r so matmuls run at 1 cycle/row."""
    return ap.bitcast(mybir.dt.float32r)


_NO_SPLIT_OPCODES = {
    "CollectiveCompute",
    "EventSemaphore",
}
_split_counter = [0]


def _split_multi_waits(nc):
    """This container's walrus accepts at most ONE sem wait per TPB
    instruction; hoist extra waits onto same-engine NOPs placed before."""
    n_split = 0
    for fn in nc.m.functions:
        for bb in fn.blocks:
            changed = False
            out = []
            for inst in bb.instructions:
                si = inst.sync_info
                if (
                    si is not None
                    and si.on_wait
                    and len(list(si.on_wait)) > 1
                    and inst.opcode not in _NO_SPLIT_OPCODES
                ):
                    waits = list(si.on_wait)
                    for w in waits[:-1]:
                        _split_counter[0] += 1
                        nop = mybir.InstNoOp(name=f"I-wsplit-{_split_counter[0]}")
                        nop.engine = inst.engine
                        nop.sync_info = mybir.SyncInfo(on_wait=[w], on_update=[])
                        out.append(nop)
                        n_split += 1
                    si.on_wait = waits[-1:]
                    changed = True
                out.append(inst)
            if changed:
                bb.instructions = out
    return n_split


def _flat2(ap):
    """[p, a, b] -> [p, a*b]"""
    return ap.rearrange("p a b -> p (a b)")


# ------------------------------------------------------------ program build
def build_program():
    nc = bass.Bass("TRN2", target_bir_lowering=False, debug=False, num_devices=N_CORES)

    dt_in = {}
    for name, shape in [
        ("qT", [D, LQ]),
        ("kwT", [D, NTOK]),
        ("vwT", [D, NTOK]),
        ("ksT", [D, NB]),
        ("vsT", [D, NB]),
        ("Wqs", [D, H * DK]),
        ("Wks", [D, H * DK]),
        ("Wvs", [D, H * DV]),
        ("Wqw", [D, H * DK]),
        ("Wkw", [D, H * DK]),
        ("Wvw", [D, H * DV]),
        ("Wfc", [D + H * DV, D]),
        ("Wfc1", [H * DV, D]),
    ]:
        dt_in[name] = nc.dram_tensor(name, shape, FR, kind="ExternalInput").ap()
    for name, shape in [
        ("bqsT", [128, 4]),
        ("bksT", [128, 4]),
        ("bqwT", [128, 4]),
        ("bkwT", [128, 4]),
        ("bvsT", [128, 4]),
        ("bfc1T", [128, 4]),
        ("bfcT", [128, 4]),
        ("bvw", [H * DV]),
    ]:
        dt_in[name] = nc.dram_tensor(name, shape, FP, kind="ExternalInput").ap()
    outT_d = nc.dram_tensor("outT", [D, LQ], FP, kind="ExternalOutput").ap()

    with tile.TileContext(nc) as tc:
        # ------------------------------------------------ persistent pools
        ppool_cm = tc.tile_pool(name="persist", bufs=1)
        ppool = ppool_cm.__enter__()
        scpool_cm = tc.tile_pool(name="scps", bufs=2, space="PSUM")
        scpool = scpool_cm.__enter__()
        s3pool_cm = tc.tile_pool(name="s3ps", bufs=3, space="PSUM")
        s3pool = s3pool_cm.__enter__()
        ewpool_cm = tc.tile_pool(name="ewp", bufs=12)
        ewpool = ewpool_cm.__enter__()
        smpool_cm = tc.tile_pool(name="small", bufs=4)
        smpool = smpool_cm.__enter__()

        ident = ppool.tile([128, 128], FP, tag="ident")

        # biases (per-partition [128,4] layouts prepared on host)
        btiles = {}
        for bn in ["bqsT", "bksT", "bqwT", "bkwT", "bvsT", "bfc1T", "bfcT"]:
            t = ppool.tile([128, 4], FP, tag=bn)
            nc.sync.dma_start(out=t[:], in_=dt_in[bn][:])
            btiles[bn] = t
        bvw_bc = ppool.tile([128, H * DV], FP, tag="bvw_bc")
        src = dt_in["bvw"]
        bcast_ap = bass.AP(src.tensor, src.offset, [[0, 128]] + [list(x) for x in src.ap])
        nc.sync.dma_start(out=bvw_bc[:], in_=bcast_ap)

        # persistent sbuf tensors
        qw_pad = ppool.tile([128, H, LQ], BF, tag="qw_pad")
        ks_sb = ppool.tile([128, 4, NB], FR, tag="ks_sb")
        attn_sb = ppool.tile([128, 4, H, NB], FP, tag="attn_sb")
        fc1T_sb = ppool.tile([128, 4, LQ], FR, tag="fc1T")
        kw_sb = ppool.tile([128, 4, NTOK], BF, tag="kw_sb")
        vw_sb = ppool.tile([128, NTC, H, DV + 1], BF, tag="vw_sb")
        ctx_acc = ppool.tile([128, 4, H * DV], FP, tag="ctx_acc")
        vs_sb = ppool.tile([NB, H * DV], FR, tag="vs_sb")
        ctx_sT = ppool.tile([128, 4, LQ], FR, tag="ctx_sT")


        # ------------------------------------------- phase 1+2: small branch
        with tc.tile_pool(name="phaseA", bufs=1) as apool, tc.tile_pool(
            name="wring", bufs=2
        ) as wpool:
            qT_sb = apool.tile([128, 4, LQ], FR, tag="qT_sb")
            ksT_sb = apool.tile([128, 4, NB], FR, tag="ksT_sb")
            vsT_sb = apool.tile([128, 4, NB], FR, tag="vsT_sb")
            qs_sb = apool.tile([128, 4, LQ], FR, tag="qs_sb")

            def load_W(wn, wdt=FR):
                t = wpool.tile([128, 4, D], wdt, tag="Wring", name=wn)
                for k in range(4):
                    nc.sync.dma_start(
                        out=t[:, k, :], in_=dt_in[wn][k * 128 : (k + 1) * 128, :]
                    )
                return t

            for k in range(4):
                nc.sync.dma_start(out=qT_sb[:, k, :], in_=dt_in["qT"][k * 128 : (k + 1) * 128, :])
                nc.sync.dma_start(out=ksT_sb[:, k, :], in_=dt_in["ksT"][k * 128 : (k + 1) * 128, :])
                nc.sync.dma_start(out=vsT_sb[:, k, :], in_=dt_in["vsT"][k * 128 : (k + 1) * 128, :])
            make_identity(nc, ident)

            # qw projection into zero-padded per-head layout: head h=2mo on
            # partitions 0:64, h=2mo+1 on 64:128 (rest stays zero so K=128
            # score matmuls against the full kw partition range are exact).
            nc.vector.memset(qw_pad[:, :, :], 0.0)
            Wqw_t = load_W("Wqw")
            for mo in range(4):
                ps = scpool.tile([128, 512], FP, tag="sc", name="qwps")
                for k in range(4):
                    nc.tensor.matmul(
                        ps[:],
                        Wqw_t[:, k, mo * 128 : (mo + 1) * 128],
                        qT_sb[:, k, :],
                        start=(k == 0),
                        stop=(k == 3),
                    )
                nc.vector.tensor_scalar_add(
                    qw_pad[0:64, 2 * mo, :], ps[0:64, :], btiles["bqwT"][0:64, mo : mo + 1]
                )
                nc.vector.tensor_scalar_add(
                    qw_pad[64:128, 2 * mo + 1, :],
                    ps[64:128, :],
                    btiles["bqwT"][64:128, mo : mo + 1],
                )
            # qs projection
            Wqs_t = load_W("Wqs")
            for mo in range(4):
                ps = scpool.tile([128, 512], FP, tag="sc", name="qsps")
                for k in range(4):
                    nc.tensor.matmul(
                        ps[:],
                        Wqs_t[:, k, mo * 128 : (mo + 1) * 128],
                        qT_sb[:, k, :],
                        start=(k == 0),
                        stop=(k == 3),
                    )
                nc.vector.tensor_scalar_add(
                    qs_sb[:, mo, :], ps[:], btiles["bqsT"][:, mo : mo + 1]
                )

            # ks projection: out [hdk(mo), nb]
            Wks_t = load_W("Wks")
            for mo in range(4):
                ps = scpool.tile([128, 512], FP, tag="sc", name="ksps")
                for k in range(4):
                    nc.tensor.matmul(
                        ps[:, 0:NB],
                        Wks_t[:, k, mo * 128 : (mo + 1) * 128],
                        ksT_sb[:, k, :],
                        start=(k == 0),
                        stop=(k == 3),
                    )
                nc.vector.tensor_scalar_add(
                    ks_sb[:, mo, :], ps[:, 0:NB], btiles["bksT"][:, mo : mo + 1]
                )

            # vs projection: out [nb, hdv]  (lhsT = vsT chunk, rhs = Wvs)
            Wvs_t = load_W("Wvs")
            ps = scpool.tile([128, 512], FP, tag="sc", name="vsps")
            for k in range(4):
                nc.tensor.matmul(
                    ps[0:NB, :],
                    vsT_sb[:, k, :],
                    Wvs_t[:, k, :],
                    start=(k == 0),
                    stop=(k == 3),
                )
            nc.scalar.activation(vs_sb[:, :], ps[0:NB, :], ACTF.Copy)

            # sentence attention, batched: (1) all score matmuls + exp,
            # (2) batched softmax on DVE, (3) transposes + ctx_sT
            ews_all = apool.tile([128, 4, H, NB], FP, tag="ews_all")
            for h in range(H):
                hp, po = h // 2, (h % 2) * 64
                ps = scpool.tile([128, 512], FP, tag="sc", name="sattps")
                for qo in range(4):
                    nc.tensor.matmul(
                        ps[:, qo * NB : (qo + 1) * NB],
                        qs_sb[po : po + 64, hp, qo * 128 : (qo + 1) * 128],
                        ks_sb[po : po + 64, hp, :],
                        start=True,
                        stop=True,
                    )
                nc.scalar.activation(
                    ews_all[:, :, h, :],
                    ps[:, 0 : 4 * NB].rearrange("p (a x) -> p a x", x=NB),
                    ACTF.Exp,
                    scale=SCALE,
                )
            for h in range(H):
                for qo in range(4):
                    den = smpool.tile([128, 1], FP, tag="den")
                    nc.vector.tensor_reduce(den[:, 0:1], ews_all[:, qo, h, :], AX, ALU.add)
                    rec = smpool.tile([128, 1], FP, tag="rec")
                    nc.vector.reciprocal(rec[:], den[:])
                    nc.vector.tensor_scalar_mul(
                        attn_sb[:, qo, h, :], ews_all[:, qo, h, :], rec[:, 0:1]
                    )

        # ---------------- phase 3+4+5: projections interleaved with attention
        stpool_cm = tc.tile_pool(name="stage", bufs=2)
        stpool = stpool_cm.__enter__()

        Wkw_sb = ppool.tile([128, 4, H * DK], FR, tag="Wkw")
        Wvw_sb = ppool.tile([128, 4, H * DV], FR, tag="Wvw")
        for k in range(4):
            nc.sync.dma_start(out=Wkw_sb[:, k, :], in_=dt_in["Wkw"][k * 128 : (k + 1) * 128, :])
            nc.sync.dma_start(out=Wvw_sb[:, k, :], in_=dt_in["Wvw"][k * 128 : (k + 1) * 128, :])
        ctx_wT = ppool.tile([128, 4, LQ], FR, tag="ctx_wT")

        # kw/vw projection stages, interleaved into head 0's block loop
        def kw_stage(sg):
            stg = stpool.tile([128, 4, 1024], FR, tag="stg", name="kwstg")
            for k in range(4):
                nc.sync.dma_start(
                    out=stg[:, k, :],
                    in_=dt_in["kwT"][k * 128 : (k + 1) * 128, sg * 1024 : (sg + 1) * 1024],
                )
            for mo in range(4):
                for j in range(2):
                    ps = scpool.tile([128, 512], FP, tag="sc", name="kwps")
                    for k in range(4):
                        nc.tensor.matmul(
                            ps[:],
                            Wkw_sb[:, k, mo * 128 : (mo + 1) * 128],
                            stg[:, k, j * 512 : (j + 1) * 512],
                            start=(k == 0),
                            stop=(k == 3),
                        )
                    nc.vector.tensor_scalar_add(
                        kw_sb[:, mo, sg * 1024 + j * 512 : sg * 1024 + (j + 1) * 512],
                        ps[:],
                        btiles["bkwT"][:, mo : mo + 1],
                    )

        def vw_stage(sg):
            stg = stpool.tile([128, 4, 1024], FR, tag="stg", name="vwstg")
            for k in range(4):
                nc.sync.dma_start(
                    out=stg[:, k, :],
                    in_=dt_in["vwT"][k * 128 : (k + 1) * 128, sg * 1024 : (sg + 1) * 1024],
                )
            for tcl in range(8):
                ps = scpool.tile([128, 512], FP, tag="sc", name="vwps")
                for k in range(4):
                    nc.tensor.matmul(
                        ps[:],
                        stg[:, k, tcl * 128 : (tcl + 1) * 128],
                        Wvw_sb[:, k, :],
                        start=(k == 0),
                        stop=(k == 3),
                    )
                nc.scalar.activation(
                    vw_sb[:, sg * 8 + tcl, :, 0:DV],
                    ps.rearrange("p (h x) -> p h x", x=DV),
                    ACTF.Copy,
                )
            nc.vector.memset(vw_sb[:, sg * 8 : (sg + 1) * 8, :, DV : DV + 1], 1.0)

        # token scores + ctx_w accumulation
        for h in range(H):
            hp, po = h // 2, (h % 2) * 64
            for npair in range(NBH // 2):
                if h == 0 and npair % 2 == 0:
                    kw_stage(npair // 2)
                    vw_stage(npair // 2)
                s3t = s3pool.tile([128, 2, 512], FP, tag="s3", name="s3t")
                s3v = s3t.rearrange("p g (qo x) -> p g qo x", x=128)
                for g in range(2):
                    n = npair * 2 + g
                    # scores [t, q] for block n (2 token chunks)
                    ews = []
                    for j in range(2):
                        tcg = 2 * n + j
                        ps_sc = scpool.tile([128, 512], FP, tag="sc", name="scps")
                        nc.tensor.matmul(
                            ps_sc[:],
                            kw_sb[:, hp, tcg * 128 : (tcg + 1) * 128],
                            qw_pad[:, h, :],
                            start=True,
                            stop=True,
                        )
                        ew_t = ewpool.tile([128, 512], BF, tag="ew", name="ew_t")
                        nc.scalar.activation(ew_t[:], ps_sc[:], ACTF.Exp, scale=SCALE)
                        ews.append(ew_t)
                    # S3: per qo, [q, dv+1] partial context for this block
                    for qo in range(4):
                        for j in range(2):
                            tcg = 2 * n + j
                            nc.tensor.matmul(
                                s3v[:, g, qo, 0 : DV + 1],
                                ews[j][:, qo * 128 : (qo + 1) * 128],
                                vw_sb[:, tcg, h, :],
                                start=(j == 0),
                                stop=(j == 1),
                            )
                # factor = attn_s / denom for the 2x4 tiles in this super
                rec_t = smpool.tile([128, 2, 4], FP, tag="rec_t")
                nc.vector.reciprocal(rec_t[:], s3v[:, :, :, DV])
                fac_t = smpool.tile([128, 2, 4], FP, tag="fac_t")
                nc.vector.tensor_mul(
                    fac_t[:],
                    rec_t[:],
                    attn_sb[:, :, h, npair * 2 : npair * 2 + 2].rearrange(
                        "p qo g -> p g qo"
                    ),
                )
                for g in range(2):
                    n = npair * 2 + g
                    for qo in range(4):
                        acc_sl = ctx_acc[:, qo, h * 64 : (h + 1) * 64]
                        if n == 0:
                            nc.vector.tensor_scalar(
                                acc_sl,
                                s3v[:, g, qo, 0:DV],
                                fac_t[:, g, qo : qo + 1],
                                None,
                                op0=ALU.mult,
                            )
                        else:
                            nc.vector.scalar_tensor_tensor(
                                acc_sl,
                                s3v[:, g, qo, 0:DV],
                                fac_t[:, g, qo : qo + 1],
                                acc_sl,
                                op0=ALU.mult,
                                op1=ALU.add,
                            )
            # + b_vw * (sum of attn_s over this core's blocks)
            for qo in range(4):
                sh = smpool.tile([128, 1], FP, tag="sh")
                nc.vector.tensor_reduce(sh[:, 0:1], attn_sb[:, qo, h, 0:NBH], AX, ALU.add)
                nc.vector.scalar_tensor_tensor(
                    ctx_acc[:, qo, h * 64 : (h + 1) * 64],
                    bvw_bc[:, h * 64 : (h + 1) * 64],
                    sh[:, 0:1],
                    ctx_acc[:, qo, h * 64 : (h + 1) * 64],
                    op0=ALU.mult,
                    op1=ALU.add,
                )
            if h % 2 == 1:
                # both heads of dv-chunk h//2 done: transpose ctx_acc slab
                dc = h // 2
                for qo in range(4):
                    ps = scpool.tile([128, 512], FP, tag="sc", name="ctps")
                    nc.tensor.transpose(
                        ps[:, 0:128], ctx_acc[:, qo, dc * 128 : (dc + 1) * 128], ident[:]
                    )
                    nc.scalar.activation(
                        ctx_wT[:, dc, qo * 128 : (qo + 1) * 128], ps[:, 0:128], ACTF.Copy
                    )
        stpool_cm.__exit__(None, None, None)

        # ---------------- phase 6: sentence-ctx tail, then final fc
        lpool_cm = tc.tile_pool(name="late", bufs=1)
        lpool = lpool_cm.__enter__()
        aspool_cm = tc.tile_pool(name="asTring", bufs=2)
        aspool = aspool_cm.__enter__()
        outT_sb = lpool.tile([128, 4, LQ], FP, tag="outT_sb")
        Wfc_sb = lpool.tile([128, 8, D], FR, tag="Wfc")
        Wfc1_sb = lpool.tile([128, 4, D], FR, tag="Wfc1l")
        for k in range(8):
            nc.sync.dma_start(out=Wfc_sb[:, k, :], in_=dt_in["Wfc"][k * 128 : (k + 1) * 128, :])
        for k in range(4):
            nc.sync.dma_start(out=Wfc1_sb[:, k, :], in_=dt_in["Wfc1"][k * 128 : (k + 1) * 128, :])

        for h in range(H):
            hp, po = h // 2, (h % 2) * 64
            asT_h = aspool.tile([NB, 4, 128], FR, tag="asT", name="asT_h")
            for qo in range(4):
                psT = scpool.tile([128, 512], FP, tag="sc", name="sattT")
                nc.tensor.transpose(psT[0:NB, 0:128], attn_sb[:, qo, h, :], ident[:])
                nc.scalar.activation(asT_h[:, qo, :], psT[0:NB, 0:128], ACTF.Copy)
            # ctx_sT [dv(h), q] = vs.T @ attn_sT (+ b_vs per-partition)
            ps = scpool.tile([128, 512], FP, tag="sc", name="ctxsps")
            nc.tensor.matmul(
                ps[0:64, :],
                vs_sb[:, h * 64 : (h + 1) * 64],
                asT_h[:, :, :],
                start=True,
                stop=True,
            )
            nc.vector.tensor_scalar_add(
                ctx_sT[po : po + 64, hp, :],
                ps[0:64, :],
                btiles["bvsT"][po : po + 64, hp : hp + 1],
            )
        # fc1: out [dm(mo), q] = Wfc1.T @ ctx_sT, scaled 0.5 (+0.5*b_fc1)
        for mo in range(4):
            ps = scpool.tile([128, 512], FP, tag="sc", name="fc1ps")
            for k in range(4):
                nc.tensor.matmul(
                    ps[:],
                    Wfc1_sb[:, k, mo * 128 : (mo + 1) * 128],
                    ctx_sT[:, k, :],
                    start=(k == 0),
                    stop=(k == 3),
                )
            nc.vector.tensor_scalar(
                fc1T_sb[:, mo, :],
                ps[:],
                0.5,
                btiles["bfc1T"][:, mo : mo + 1],
                op0=ALU.mult,
                op1=ALU.add,
            )
        for mo in range(4):
            ps = scpool.tile([128, 512], FP, tag="sc", name="fcps")
            for cc in range(4):
                nc.tensor.matmul(
                    ps[:],
                    Wfc_sb[:, cc, mo * 128 : (mo + 1) * 128],
                    fc1T_sb[:, cc, :],
                    start=(cc == 0),
                    stop=False,
                )
            for cc in range(4):
                nc.tensor.matmul(
                    ps[:],
                    Wfc_sb[:, 4 + cc, mo * 128 : (mo + 1) * 128],
                    ctx_wT[:, cc, :],
                    start=False,
                    stop=(cc == 3),
                )
            nc.vector.tensor_scalar_add(
                outT_sb[:, mo, :], ps[:], btiles["bfcT"][:, mo : mo + 1]
            )
            nc.sync.dma_start(out=outT_d[mo * 128 : (mo + 1) * 128, :], in_=outT_sb[:, mo, :])

        aspool_cm.__exit__(None, None, None)
        lpool_cm.__exit__(None, None, None)
        smpool_cm.__exit__(None, None, None)
        ewpool_cm.__exit__(None, None, None)
        s3pool_cm.__exit__(None, None, None)
        scpool_cm.__exit__(None, None, None)
        ppool_cm.__exit__(None, None, None)

    ns = _split_multi_waits(nc)
    print(f"[kernel] split {ns} extra sem waits onto NOPs", file=sys.stderr)
    return nc


_NC_CACHE = None


def _get_nc():
    global _NC_CACHE
    if _NC_CACHE is None:
        _NC_CACHE = build_program()
    return _NC_CACHE


def make_in_maps(inputs):
    f = lambda x: np.ascontiguousarray(np.asarray(x, dtype=np.float32))
    q, k_w, v_w, k_s, v_s = (f(inputs[n]) for n in ["q", "k_w", "v_w", "k_s", "v_s"])
    W = {n: f(inputs[n]) for n in inputs if n.startswith(("W_", "b_"))}

    def bT(v, scale=1.0):
        return np.ascontiguousarray((v * scale).reshape(4, 128).T)

    shared = {
        "Wqs": W["W_qs"], "Wks": W["W_ks"], "Wvs": W["W_vs"],
        "Wqw": W["W_qw"], "Wkw": W["W_kw"], "Wvw": W["W_vw"],
        "Wfc1": W["W_fc1"], "Wfc": W["W_fc"],
        "bqsT": bT(W["b_qs"]), "bksT": bT(W["b_ks"]), "bqwT": bT(W["b_qw"]),
        "bkwT": bT(W["b_kw"]), "bvsT": bT(W["b_vs"]),
        "bfc1T": bT(W["b_fc1"], 0.5), "bfcT": bT(W["b_fc"], 0.5),
        "bvw": W["b_vw"],
    }
    in_maps = []
    for c in range(N_CORES):
        b, half = divmod(c, 2)
        blk = slice(half * NBH, half * NBH + NBH)
        ks_r = np.roll(k_s[b], -half * NBH, axis=0)
        vs_r = np.roll(v_s[b], -half * NBH, axis=0)
        m = dict(shared)
        m["qT"] = np.ascontiguousarray(q[b].T)
        m["kwT"] = np.ascontiguousarray(k_w[b, blk].reshape(NTOK, D).T)
        m["vwT"] = np.ascontiguousarray(v_w[b, blk].reshape(NTOK, D).T)
        m["ksT"] = np.ascontiguousarray(ks_r.T)
        m["vsT"] = np.ascontiguousarray(vs_r.T)
        in_maps.append(m)
    return in_maps


def run_cores(inputs, trace=False):
    nc = _get_nc()
    in_maps = make_in_maps(inputs)
    res = run_bass_kernel_spmd(nc, in_maps, list(range(N_CORES)), trace=trace)
    return res


def assemble(res):
    out = np.empty((B, LQ, D), dtype=np.float32)
    for b in range(B):
        out[b] = (res.results[2 * b]["outT"] + res.results[2 * b + 1]["outT"]).T
    return out


def kernel(**inputs) -> np.ndarray:
    res = run_cores(inputs, trace=False)
    return assemble(res)


if __name__ == "__main__":
    import reference

    inp = {k: np.asarray(v) for k, v in reference.setup_inputs().items()}
    out = kernel(**inp)
    exp = np.asarray(reference.reference(**inp))
    err = np.abs(out - exp).max() / np.abs(exp).max()
    print("max rel err:", err)


# revision 22
# speedup vs baseline: 1.1450x; 1.1450x over previous
"""
Trainium2 Bass kernel for nn_MultiHeadHierarchicalAttention.

Sharding: 8 cores = (batch b in 0..3) x (block-half in 0..1).
Each core handles one batch and 16 of the 32 blocks for the token-level
attention; the (small) sentence-level attention + fc1 branch is computed
redundantly on both cores of a batch, scaled by 0.5, and the host sums the
two per-batch partial outputs (the final fc is linear, so partial ctx_w
contributions simply add).

Device layouts (per core, partition dim first):
  qT   [D, LQ]      kwT/vwT [D, 4096]   ksT/vsT [D, 32] (block-rolled)
  projections keep features on partitions (kw) or tokens on partitions (vw)
  token scores are computed as [t, q] tiles (K=dk=64), exp'd on ACT into
  bf16, and consumed per block by S3 matmuls producing [q, dv+1] partials
  (the +1 "ones" column of vw gives the softmax denominator for free).
  The sentence-attention factor attn_s/denom is applied with fused DVE
  scalar_tensor_tensor accumulation. ctx_w is PE-transposed at the end and
  fused into the final fc, which is emitted as outT [D, LQ] (host transposes).
"""

import sys

sys.path.insert(0, "/opt/trn_rl_repo")

import numpy as np
import concourse.bass as bass
import concourse.tile as tile
from concourse import mybir
from concourse.bass_utils import run_bass_kernel_spmd
from concourse.vector_clock import ScopedClock
from concourse.masks import make_identity

# ---------------------------------------------------------------- constants
B, LQ, NB, NT = 4, 512, 32, 256
D, H, DK, DV = 512, 8, 64, 64
NBH = NB // 2  # blocks per core
NTOK = NBH * NT  # tokens per core = 4096
NTC = NTOK // 128  # 32 token chunks of 128
SCALE = 0.125
FP = mybir.dt.float32
FR = mybir.dt.float32r
BF = mybir.dt.bfloat16
N_CORES = 8

AX = mybir.AxisListType.X
ALU = mybir.AluOpType
ACTF = mybir.ActivationFunctionType


# --------------------------------------------------------- drain workaround
def _patched_drain_and_barrier(self, tick_clock, wait_clock):
    # walrus in this container rejects >1 sem wait on a single TPB_CTRL
    # instruction ("Too many sync wait commands"); split the kernel-tail
    # drain waits across one-wait NOPs.
    nop_inst = self.nc.sync.nop(nofuse=True)
    wait_clock.add_sem_waits(nop_inst.ins, ScopedClock({None: tick_clock.global_clock}))
    waits = list(nop_inst.ins.sync_info.on_wait or [])
    if len(waits) > 1:
        nop_inst.ins.sync_info.on_wait = waits[:1]
        rest = waits[1:]
        while rest:
            extra = self.nc.sync.nop(nofuse=True)
            if extra.ins.sync_info is None:
                extra.ins.sync_info = mybir.SyncInfo(on_wait=[], on_update=[])
            extra.ins.sync_info.on_wait = rest[:1]
            rest = rest[1:]
    self.nc.sync.drain()
    self.nc.all_engine_barrier()
    assert self.sems is not None
    popped = self.nc._tile_sem_poison_stack.pop()
    assert popped is self._sem_poison
    self.nc.clear_and_free_semaphores(list(self.sems.allocated().values()))
    self.nc.all_engine_barrier()


tile.TileContext._drain_and_barrier = _patched_drain_and_barrier


def _r(ap):
    """View an f32 AP as float32r so matmuls run at 1 cycle/row."""
    return ap.bitcast(mybir.dt.float32r)


_NO_SPLIT_OPCODES = {
    "CollectiveCompute",
    "EventSemaphore",
}
_split_counter = [0]


def _split_multi_waits(nc):
    """This container's walrus accepts at most ONE sem wait per TPB
    instruction; hoist extra waits onto same-engine NOPs placed before."""
    n_split = 0
    for fn in nc.m.functions:
        for bb in fn.blocks:
            changed = False
            out = []
            for inst in bb.instructions:
                si = inst.sync_info
                if (
                    si is not None
                    and si.on_wait
                    and len(list(si.on_wait)) > 1
                    and inst.opcode not in _NO_SPLIT_OPCODES
                ):
                    waits = list(si.on_wait)
                    for w in waits[:-1]:
                        _split_counter[0] += 1
                        nop = mybir.InstNoOp(name=f"I-wsplit-{_split_counter[0]}")
                        nop.engine = inst.engine
                        nop.sync_info = mybir.SyncInfo(on_wait=[w], on_update=[])
                        out.append(nop)
                        n_split += 1
                    si.on_wait = waits[-1:]
                    changed = True
                out.append(inst)
            if changed:
                bb.instructions = out
    return n_split


def _flat2(ap):
    """[p, a, b] -> [p, a*b]"""
    return ap.rearrange("p a b -> p (a b)")


# ------------------------------------------------------------ program build
def build_program():
    nc = bass.Bass("TRN2", target_bir_lowering=False, debug=False, num_devices=N_CORES)

    dt_in = {}
    for name, shape in [
        ("qT", [D, LQ]),
        ("kwT", [D, NTOK]),
        ("vwT", [D, NTOK]),
        ("ksT", [D, NB]),
        ("vsT", [D, NB]),
        ("Wqs", [D, H * DK]),
        ("Wks", [D, H * DK]),
        ("Wvs", [D, H * DV]),
        ("Wqw", [D, H * DK]),
        ("Wkw", [D, H * DK]),
        ("Wvw", [D, H * DV]),
        ("Wfc", [D + H * DV, D]),
        ("Wfc1", [H * DV, D]),
    ]:
        dt_in[name] = nc.dram_tensor(name, shape, FR, kind="ExternalInput").ap()
    for name, shape in [
        ("bqsT", [128, 4]),
        ("bksT", [128, 4]),
        ("bqwT", [128, 4]),
        ("bkwT", [128, 4]),
        ("bvsT", [128, 4]),
        ("bfc1T", [128, 4]),
        ("bfcT", [128, 4]),
        ("bvw", [H * DV]),
    ]:
        dt_in[name] = nc.dram_tensor(name, shape, FP, kind="ExternalInput").ap()
    outT_d = nc.dram_tensor("outT", [D, LQ], FP, kind="ExternalOutput").ap()

    with tile.TileContext(nc) as tc:
        # ------------------------------------------------ persistent pools
        ppool_cm = tc.tile_pool(name="persist", bufs=1)
        ppool = ppool_cm.__enter__()
        scpool_cm = tc.tile_pool(name="scps", bufs=2, space="PSUM")
        scpool = scpool_cm.__enter__()
        s3pool_cm = tc.tile_pool(name="s3ps", bufs=2, space="PSUM")
        s3pool = s3pool_cm.__enter__()
        ewpool_cm = tc.tile_pool(name="ewp", bufs=8)
        ewpool = ewpool_cm.__enter__()
        smpool_cm = tc.tile_pool(name="small", bufs=4)
        smpool = smpool_cm.__enter__()

        ident = ppool.tile([128, 128], FP, tag="ident")

        # biases (per-partition [128,4] layouts prepared on host)
        btiles = {}
        for bn in ["bqsT", "bksT", "bqwT", "bkwT", "bvsT", "bfc1T", "bfcT"]:
            t = ppool.tile([128, 4], FP, tag=bn)
            nc.sync.dma_start(out=t[:], in_=dt_in[bn][:])
            btiles[bn] = t
        bvw_bc = ppool.tile([128, H * DV], FP, tag="bvw_bc")
        src = dt_in["bvw"]
        bcast_ap = bass.AP(src.tensor, src.offset, [[0, 128]] + [list(x) for x in src.ap])
        nc.sync.dma_start(out=bvw_bc[:], in_=bcast_ap)

        # persistent sbuf tensors
        qw_pad = ppool.tile([128, H, LQ], BF, tag="qw_pad")
        ks_sb = ppool.tile([128, 4, NB], FR, tag="ks_sb")
        attn_sb = ppool.tile([128, 4, H, NB], FP, tag="attn_sb")
        fc1T_sb = ppool.tile([128, 4, LQ], FR, tag="fc1T")
        kw_sb = ppool.tile([128, 4, NTOK], BF, tag="kw_sb")
        vw_sb = ppool.tile([128, NTC, H, DV + 1], BF, tag="vw_sb")
        ctx_acc = ppool.tile([128, 4, H * DV], FP, tag="ctx_acc")
        vs_sb = ppool.tile([NB, H * DV], FR, tag="vs_sb")
        ctx_sT = ppool.tile([128, 4, LQ], FR, tag="ctx_sT")


        # ------------------------------------------- phase 1+2: small branch
        with tc.tile_pool(name="phaseA", bufs=1) as apool, tc.tile_pool(
            name="wring", bufs=2
        ) as wpool:
            qT_sb = apool.tile([128, 4, LQ], FR, tag="qT_sb")
            ksT_sb = apool.tile([128, 4, NB], FR, tag="ksT_sb")
            vsT_sb = apool.tile([128, 4, NB], FR, tag="vsT_sb")
            qs_sb = apool.tile([128, 4, LQ], FR, tag="qs_sb")

            def load_W(wn, wdt=FR):
                t = wpool.tile([128, 4, D], wdt, tag="Wring", name=wn)
                for k in range(4):
                    nc.sync.dma_start(
                        out=t[:, k, :], in_=dt_in[wn][k * 128 : (k + 1) * 128, :]
                    )
                return t

            for k in range(4):
                nc.sync.dma_start(out=qT_sb[:, k, :], in_=dt_in["qT"][k * 128 : (k + 1) * 128, :])
                nc.sync.dma_start(out=ksT_sb[:, k, :], in_=dt_in["ksT"][k * 128 : (k + 1) * 128, :])
                nc.sync.dma_start(out=vsT_sb[:, k, :], in_=dt_in["vsT"][k * 128 : (k + 1) * 128, :])
            make_identity(nc, ident)

            # qw projection into zero-padded per-head layout: head h=2mo on
            # partitions 0:64, h=2mo+1 on 64:128 (rest stays zero so K=128
            # score matmuls against the full kw partition range are exact).
            nc.vector.memset(qw_pad[:, :, :], 0.0)
            Wqw_t = load_W("Wqw")
            for mo in range(4):
                ps = scpool.tile([128, 512], FP, tag="sc", name="qwps")
                for k in range(4):
                    nc.tensor.matmul(
                        ps[:],
                        Wqw_t[:, k, mo * 128 : (mo + 1) * 128],
                        qT_sb[:, k, :],
                        start=(k == 0),
                        stop=(k == 3),
                    )
                nc.vector.tensor_scalar_add(
                    qw_pad[0:64, 2 * mo, :], ps[0:64, :], btiles["bqwT"][0:64, mo : mo + 1]
                )
                nc.vector.tensor_scalar_add(
                    qw_pad[64:128, 2 * mo + 1, :],
                    ps[64:128, :],
                    btiles["bqwT"][64:128, mo : mo + 1],
                )
            # qs projection
            Wqs_t = load_W("Wqs")
            for mo in range(4):
                ps = scpool.tile([128, 512], FP, tag="sc", name="qsps")
                for k in range(4):
                    nc.tensor.matmul(
                        ps[:],
                        Wqs_t[:, k, mo * 128 : (mo + 1) * 128],
                        qT_sb[:, k, :],
                        start=(k == 0),
                        stop=(k == 3),
                    )
                nc.vector.tensor_scalar_add(
                    qs_sb[:, mo, :], ps[:], btiles["bqsT"][:, mo : mo + 1]
                )

            # ks projection: out [hdk(mo), nb]
            Wks_t = load_W("Wks")
            for mo in range(4):
                ps = scpool.tile([128, 512], FP, tag="sc", name="ksps")
                for k in range(4):
                    nc.tensor.matmul(
                        ps[:, 0:NB],
                        Wks_t[:, k, mo * 128 : (mo + 1) * 128],
                        ksT_sb[:, k, :],
                        start=(k == 0),
                        stop=(k == 3),
                    )
                nc.vector.tensor_scalar_add(
                    ks_sb[:, mo, :], ps[:, 0:NB], btiles["bksT"][:, mo : mo + 1]
                )

            # vs projection: out [nb, hdv]  (lhsT = vsT chunk, rhs = Wvs)
            Wvs_t = load_W("Wvs")
            ps = scpool.tile([128, 512], FP, tag="sc", name="vsps")
            for k in range(4):
                nc.tensor.matmul(
                    ps[0:NB, :],
                    vsT_sb[:, k, :],
                    Wvs_t[:, k, :],
                    start=(k == 0),
                    stop=(k == 3),
                )
            nc.scalar.activation(vs_sb[:, :], ps[0:NB, :], ACTF.Copy)

            # sentence attention, batched: (1) all score matmuls + exp,
            # (2) batched softmax on DVE, (3) transposes + ctx_sT
            ews_all = apool.tile([128, 4, H, NB], FP, tag="ews_all")
            for h in range(H):
                hp, po = h // 2, (h % 2) * 64
                ps = scpool.tile([128, 512], FP, tag="sc", name="sattps")
                for qo in range(4):
                    nc.tensor.matmul(
                        ps[:, qo * NB : (qo + 1) * NB],
                        qs_sb[po : po + 64, hp, qo * 128 : (qo + 1) * 128],
                        ks_sb[po : po + 64, hp, :],
                        start=True,
                        stop=True,
                    )
                nc.scalar.activation(
                    ews_all[:, :, h, :],
                    ps[:, 0 : 4 * NB].rearrange("p (a x) -> p a x", x=NB),
                    ACTF.Exp,
                    scale=SCALE,
                )
            for h in range(H):
                for qo in range(4):
                    den = smpool.tile([128, 1], FP, tag="den")
                    nc.vector.tensor_reduce(den[:, 0:1], ews_all[:, qo, h, :], AX, ALU.add)
                    rec = smpool.tile([128, 1], FP, tag="rec")
                    nc.vector.reciprocal(rec[:], den[:])
                    nc.vector.tensor_scalar_mul(
                        attn_sb[:, qo, h, :], ews_all[:, qo, h, :], rec[:, 0:1]
                    )

        # ---------------- phase 3+4+5: projections interleaved with attention
        stpool_cm = tc.tile_pool(name="stage", bufs=2)
        stpool = stpool_cm.__enter__()

        Wkw_sb = ppool.tile([128, 4, H * DK], FR, tag="Wkw")
        Wvw_sb = ppool.tile([128, 4, H * DV], FR, tag="Wvw")
        for k in range(4):
            nc.sync.dma_start(out=Wkw_sb[:, k, :], in_=dt_in["Wkw"][k * 128 : (k + 1) * 128, :])
            nc.sync.dma_start(out=Wvw_sb[:, k, :], in_=dt_in["Wvw"][k * 128 : (k + 1) * 128, :])
        ctx_wT = ppool.tile([128, 4, LQ], FR, tag="ctx_wT")

        # kw/vw projection stages, interleaved into head 0's block loop
        def kw_stage(sg):
            stg = stpool.tile([128, 4, 1024], FR, tag="stg", name="kwstg")
            for k in range(4):
                nc.sync.dma_start(
                    out=stg[:, k, :],
                    in_=dt_in["kwT"][k * 128 : (k + 1) * 128, sg * 1024 : (sg + 1) * 1024],
                )
            for mo in range(4):
                ps = scpool.tile([128, 2, 512], FP, tag="sc", name="kwps")
                for j in range(2):
                    for k in range(4):
                        nc.tensor.matmul(
                            ps[:, j, :],
                            Wkw_sb[:, k, mo * 128 : (mo + 1) * 128],
                            stg[:, k, j * 512 : (j + 1) * 512],
                            start=(k == 0),
                            stop=(k == 3),
                        )
                nc.vector.tensor_scalar_add(
                    kw_sb[:, mo, sg * 1024 : (sg + 1) * 1024],
                    _flat2(ps),
                    btiles["bkwT"][:, mo : mo + 1],
                )

        def vw_stage(sg):
            stg = stpool.tile([128, 4, 1024], FR, tag="stg", name="vwstg")
            for k in range(4):
                nc.sync.dma_start(
                    out=stg[:, k, :],
                    in_=dt_in["vwT"][k * 128 : (k + 1) * 128, sg * 1024 : (sg + 1) * 1024],
                )
            for tp in range(4):
                ps = scpool.tile([128, 2, 512], FP, tag="sc", name="vwps")
                for j in range(2):
                    tcl = tp * 2 + j
                    for k in range(4):
                        nc.tensor.matmul(
                            ps[:, j, :],
                            stg[:, k, tcl * 128 : (tcl + 1) * 128],
                            Wvw_sb[:, k, :],
                            start=(k == 0),
                            stop=(k == 3),
                        )
                tc0 = sg * 8 + tp * 2
                nc.scalar.activation(
                    vw_sb[:, tc0 : tc0 + 2, :, 0:DV],
                    ps.rearrange("p a (h x) -> p a h x", x=DV),
                    ACTF.Copy,
                )
            nc.vector.memset(vw_sb[:, sg * 8 : (sg + 1) * 8, :, DV : DV + 1], 1.0)

        # token scores + ctx_w accumulation
        for h in range(H):
            hp, po = h // 2, (h % 2) * 64
            for npair in range(NBH // 2):
                if h == 0 and npair % 2 == 0:
                    kw_stage(npair // 2)
                    vw_stage(npair // 2)
                s3t = s3pool.tile([128, 2, 512], FP, tag="s3", name="s3t")
                s3v = s3t.rearrange("p g (qo x) -> p g qo x", x=128)
                for g in range(2):
                    n = npair * 2 + g
                    # scores [t, q] for block n (2 token chunks)
                    ps_sc = scpool.tile([128, 2, 512], FP, tag="sc", name="scps")
                    for j in range(2):
                        tcg = 2 * n + j
                        nc.tensor.matmul(
                            ps_sc[:, j, :],
                            kw_sb[:, hp, tcg * 128 : (tcg + 1) * 128],
                            qw_pad[:, h, :],
                            start=True,
                            stop=True,
                        )
                    ew_t = ewpool.tile([128, 2, 512], BF, tag="ew", name="ew_t")
                    nc.scalar.activation(_flat2(ew_t), _flat2(ps_sc), ACTF.Exp, scale=SCALE)
                    # S3: per qo, [q, dv+1] partial context for this block
                    for qo in range(4):
                        for j in range(2):
                            tcg = 2 * n + j
                            nc.tensor.matmul(
                                s3v[:, g, qo, 0 : DV + 1],
                                ew_t[:, j, qo * 128 : (qo + 1) * 128],
                                vw_sb[:, tcg, h, :],
                                start=(j == 0),
                                stop=(j == 1),
                            )
                # factor = attn_s / denom for the 2x4 tiles in this super
                rec_t = smpool.tile([128, 2, 4], FP, tag="rec_t")
                nc.vector.reciprocal(rec_t[:], s3v[:, :, :, DV])
                fac_t = smpool.tile([128, 2, 4], FP, tag="fac_t")
                nc.vector.tensor_mul(
                    fac_t[:],
                    rec_t[:],
                    attn_sb[:, :, h, npair * 2 : npair * 2 + 2].rearrange(
                        "p qo g -> p g qo"
                    ),
                )
                for g in range(2):
                    n = npair * 2 + g
                    for qo in range(4):
                        acc_sl = ctx_acc[:, qo, h * 64 : (h + 1) * 64]
                        if n == 0:
                            nc.vector.tensor_scalar(
                                acc_sl,
                                s3v[:, g, qo, 0:DV],
                                fac_t[:, g, qo : qo + 1],
                                None,
                                op0=ALU.mult,
                            )
                        else:
                            nc.vector.scalar_tensor_tensor(
                                acc_sl,
                                s3v[:, g, qo, 0:DV],
                                fac_t[:, g, qo : qo + 1],
                                acc_sl,
                                op0=ALU.mult,
                                op1=ALU.add,
                            )
            # + b_vw * (sum of attn_s over this core's blocks)
            for qo in range(4):
                sh = smpool.tile([128, 1], FP, tag="sh")
                nc.vector.tensor_reduce(sh[:, 0:1], attn_sb[:, qo, h, 0:NBH], AX, ALU.add)
                nc.vector.scalar_tensor_tensor(
                    ctx_acc[:, qo, h * 64 : (h + 1) * 64],
                    bvw_bc[:, h * 64 : (h + 1) * 64],
                    sh[:, 0:1],
                    ctx_acc[:, qo, h * 64 : (h + 1) * 64],
                    op0=ALU.mult,
                    op1=ALU.add,
                )
            if h % 2 == 1:
                # both heads of dv-chunk h//2 done: transpose ctx_acc slab
                dc = h // 2
                for qo in range(4):
                    ps = scpool.tile([128, 512], FP, tag="sc", name="ctps")
                    nc.tensor.transpose(
                        ps[:, 0:128], ctx_acc[:, qo, dc * 128 : (dc + 1) * 128], ident[:]
                    )
                    nc.scalar.activation(
                        ctx_wT[:, dc, qo * 128 : (qo + 1) * 128], ps[:, 0:128], ACTF.Copy
                    )
        stpool_cm.__exit__(None, None, None)

        # ---------------- phase 6: sentence-ctx tail, then final fc
        lpool_cm = tc.tile_pool(name="late", bufs=1)
        lpool = lpool_cm.__enter__()
        aspool_cm = tc.tile_pool(name="asTring", bufs=2)
        aspool = aspool_cm.__enter__()
        outT_sb = lpool.tile([128, 4, LQ], FP, tag="outT_sb")
        Wfc_sb = lpool.tile([128, 8, D], FR, tag="Wfc")
        Wfc1_sb = lpool.tile([128, 4, D], FR, tag="Wfc1l")
        for k in range(8):
            nc.sync.dma_start(out=Wfc_sb[:, k, :], in_=dt_in["Wfc"][k * 128 : (k + 1) * 128, :])
        for k in range(4):
            nc.sync.dma_start(out=Wfc1_sb[:, k, :], in_=dt_in["Wfc1"][k * 128 : (k + 1) * 128, :])

        for h in range(H):
            hp, po = h // 2, (h % 2) * 64
            asT_h = aspool.tile([NB, 4, 128], FR, tag="asT", name="asT_h")
            for qo in range(4):
                psT = scpool.tile([128, 512], FP, tag="sc", name="sattT")
                nc.tensor.transpose(psT[0:NB, 0:128], attn_sb[:, qo, h, :], ident[:])
                nc.scalar.activation(asT_h[:, qo, :], psT[0:NB, 0:128], ACTF.Copy)
            # ctx_sT [dv(h), q] = vs.T @ attn_sT (+ b_vs per-partition)
            ps = scpool.tile([128, 512], FP, tag="sc", name="ctxsps")
            nc.tensor.matmul(
                ps[0:64, :],
                vs_sb[:, h * 64 : (h + 1) * 64],
                asT_h[:, :, :],
                start=True,
                stop=True,
            )
            nc.vector.tensor_scalar_add(
                ctx_sT[po : po + 64, hp, :],
                ps[0:64, :],
                btiles["bvsT"][po : po + 64, hp : hp + 1],
            )
        # fc1: out [dm(mo), q] = Wfc1.T @ ctx_sT, scaled 0.5 (+0.5*b_fc1)
        for mo in range(4):
            ps = scpool.tile([128, 512], FP, tag="sc", name="fc1ps")
            for k in range(4):
                nc.tensor.matmul(
                    ps[:],
                    Wfc1_sb[:, k, mo * 128 : (mo + 1) * 128],
                    ctx_sT[:, k, :],
                    start=(k == 0),
                    stop=(k == 3),
                )
            nc.vector.tensor_scalar(
                fc1T_sb[:, mo, :],
                ps[:],
                0.5,
                btiles["bfc1T"][:, mo : mo + 1],
                op0=ALU.mult,
                op1=ALU.add,
            )
        for mo in range(4):
            ps = scpool.tile([128, 512], FP, tag="sc", name="fcps")
            for cc in range(4):
                nc.tensor.matmul(
                    ps[:],
                    Wfc_sb[:, cc, mo * 128 : (mo + 1) * 128],
                    fc1T_sb[:, cc, :],
                    start=(cc == 0),
                    stop=False,
                )
            for cc in range(4):
                nc.tensor.matmul(
                    ps[:],
                    Wfc_sb[:, 4 + cc, mo * 128 : (mo + 1) * 128],
                    ctx_wT[:, cc, :],
                    start=False,
                    stop=(cc == 3),
                )
            nc.vector.tensor_scalar_add(
                outT_sb[:, mo, :], ps[:], btiles["bfcT"][:, mo : mo + 1]
            )
            nc.sync.dma_start(out=outT_d[mo * 128 : (mo + 1) * 128, :], in_=outT_sb[:, mo, :])

        aspool_cm.__exit__(None, None, None)
        lpool_cm.__exit__(None, None, None)
        smpool_cm.__exit__(None, None, None)
        ewpool_cm.__exit__(None, None, None)
        s3pool_cm.__exit__(None, None, None)
        scpool_cm.__exit__(None, None, None)
        ppool_cm.__exit__(None, None, None)

    ns = _split_multi_waits(nc)
    print(f"[kernel] split {ns} extra sem waits onto NOPs", file=sys.stderr)
    return nc


_NC_CACHE = None


def _get_nc():
    global _NC_CACHE
    if _NC_CACHE is None:
        _NC_CACHE = build_program()
    return _NC_CACHE


def make_in_maps(inputs):
    f = lambda x: np.ascontiguousarray(np.asarray(x, dtype=np.float32))
    q, k_w, v_w, k_s, v_s = (f(inputs[n]) for n in ["q", "k_w", "v_w", "k_s", "v_s"])
    W = {n: f(inputs[n]) for n in inputs if n.startswith(("W_", "b_"))}

    def bT(v, scale=1.0):
        return np.ascontiguousarray((v * scale).reshape(4, 128).T)

    shared = {
        "Wqs": W["W_qs"], "Wks": W["W_ks"], "Wvs": W["W_vs"],
        "Wqw": W["W_qw"], "Wkw": W["W_kw"], "Wvw": W["W_vw"],
        "Wfc1": W["W_fc1"], "Wfc": W["W_fc"],
        "bqsT": bT(W["b_qs"]), "bksT": bT(W["b_ks"]), "bqwT": bT(W["b_qw"]),
        "bkwT": bT(W["b_kw"]), "bvsT": bT(W["b_vs"]),
        "bfc1T": bT(W["b_fc1"], 0.5), "bfcT": bT(W["b_fc"], 0.5),
        "bvw": W["b_vw"],
    }
    in_maps = []
    for c in range(N_CORES):
        b, half = divmod(c, 2)
        blk = slice(half * NBH, half * NBH + NBH)
        ks_r = np.roll(k_s[b], -half * NBH, axis=0)
        vs_r = np.roll(v_s[b], -half * NBH, axis=0)
        m = dict(shared)
        m["qT"] = np.ascontiguousarray(q[b].T)
        m["kwT"] = np.ascontiguousarray(k_w[b, blk].reshape(NTOK, D).T)
        m["vwT"] = np.ascontiguousarray(v_w[b, blk].reshape(NTOK, D).T)
        m["ksT"] = np.ascontiguousarray(ks_r.T)
        m["vsT"] = np.ascontiguousarray(vs_r.T)
        in_maps.append(m)
    return in_maps


def run_cores(inputs, trace=False):
    nc = _get_nc()
    in_maps = make_in_maps(inputs)
    res = run_bass_kernel_spmd(nc, in_maps, list(range(N_CORES)), trace=trace)
    return res


def assemble(res):
    out = np.empty((B, LQ, D), dtype=np.float32)
    for b in range(B):
        out[b] = (res.results[2 * b]["outT"] + res.results[2 * b + 1]["outT"]).T
    return out


def kernel(**inputs) -> np.ndarray:
    res = run_cores(inputs, trace=False)
    return assemble(res)


if __name__ == "__main__":
    import reference

    inp = {k: np.asarray(v) for k, v in reference.setup_inputs().items()}
    out = kernel(**inp)
    exp = np.asarray(reference.reference(**inp))
    err = np.abs(out - exp).max() / np.abs(exp).max()
    print("max rel err:", err)


# revision 23
# speedup vs baseline: 1.1563x; 1.0099x over previous
"""
Trainium2 Bass kernel for nn_MultiHeadHierarchicalAttention.

Sharding: 8 cores = (batch b in 0..3) x (block-half in 0..1).
Each core handles one batch and 16 of the 32 blocks for the token-level
attention; the (small) sentence-level attention + fc1 branch is computed
redundantly on both cores of a batch, scaled by 0.5, and the host sums the
two per-batch partial outputs (the final fc is linear, so partial ctx_w
contributions simply add).

Device layouts (per core, partition dim first):
  qT   [D, LQ]      kwT/vwT [D, 4096]   ksT/vsT [D, 32] (block-rolled)
  projections keep features on partitions (kw) or tokens on partitions (vw)
  token scores are computed as [t, q] tiles (K=dk=64), exp'd on ACT into
  bf16, and consumed per block by S3 matmuls producing [q, dv+1] partials
  (the +1 "ones" column of vw gives the softmax denominator for free).
  The sentence-attention factor attn_s/denom is applied with fused DVE
  scalar_tensor_tensor accumulation. ctx_w is PE-transposed at the end and
  fused into the final fc, which is emitted as outT [D, LQ] (host transposes).
"""

import sys

sys.path.insert(0, "/opt/trn_rl_repo")

import numpy as np
import concourse.bass as bass
import concourse.tile as tile
from concourse import mybir
from concourse.bass_utils import run_bass_kernel_spmd
from concourse.vector_clock import ScopedClock
from concourse.masks import make_identity

# ---------------------------------------------------------------- constants
B, LQ, NB, NT = 4, 512, 32, 256
D, H, DK, DV = 512, 8, 64, 64
NBH = NB // 2  # blocks per core
NTOK = NBH * NT  # tokens per core = 4096
NTC = NTOK // 128  # 32 token chunks of 128
SCALE = 0.125
FP = mybir.dt.float32
FR = mybir.dt.float32r
BF = mybir.dt.bfloat16
N_CORES = 8

AX = mybir.AxisListType.X
ALU = mybir.AluOpType
ACTF = mybir.ActivationFunctionType


# --------------------------------------------------------- drain workaround
def _patched_drain_and_barrier(self, tick_clock, wait_clock):
    # walrus in this container rejects >1 sem wait on a single TPB_CTRL
    # instruction ("Too many sync wait commands"); split the kernel-tail
    # drain waits across one-wait NOPs.
    nop_inst = self.nc.sync.nop(nofuse=True)
    wait_clock.add_sem_waits(nop_inst.ins, ScopedClock({None: tick_clock.global_clock}))
    waits = list(nop_inst.ins.sync_info.on_wait or [])
    if len(waits) > 1:
        nop_inst.ins.sync_info.on_wait = waits[:1]
        rest = waits[1:]
        while rest:
            extra = self.nc.sync.nop(nofuse=True)
            if extra.ins.sync_info is None:
                extra.ins.sync_info = mybir.SyncInfo(on_wait=[], on_update=[])
            extra.ins.sync_info.on_wait = rest[:1]
            rest = rest[1:]
    self.nc.sync.drain()
    self.nc.all_engine_barrier()
    assert self.sems is not None
    popped = self.nc._tile_sem_poison_stack.pop()
    assert popped is self._sem_poison
    self.nc.clear_and_free_semaphores(list(self.sems.allocated().values()))
    self.nc.all_engine_barrier()


tile.TileContext._drain_and_barrier = _patched_drain_and_barrier


def _r(ap):
    """View an f32 AP as float32r so matmuls run at 1 cycle/row."""
    return ap.bitcast(mybir.dt.float32r)


_NO_SPLIT_OPCODES = {
    "CollectiveCompute",
    "EventSemaphore",
}
_split_counter = [0]


def _split_multi_waits(nc):
    """This container's walrus accepts at most ONE sem wait per TPB
    instruction; hoist extra waits onto same-engine NOPs placed before."""
    n_split = 0
    for fn in nc.m.functions:
        for bb in fn.blocks:
            changed = False
            out = []
            for inst in bb.instructions:
                si = inst.sync_info
                if (
                    si is not None
                    and si.on_wait
                    and len(list(si.on_wait)) > 1
                    and inst.opcode not in _NO_SPLIT_OPCODES
                ):
                    waits = list(si.on_wait)
                    for w in waits[:-1]:
                        _split_counter[0] += 1
                        nop = mybir.InstNoOp(name=f"I-wsplit-{_split_counter[0]}")
                        nop.engine = inst.engine
                        nop.sync_info = mybir.SyncInfo(on_wait=[w], on_update=[])
                        out.append(nop)
                        n_split += 1
                    si.on_wait = waits[-1:]
                    changed = True
                out.append(inst)
            if changed:
                bb.instructions = out
    return n_split


def _flat2(ap):
    """[p, a, b] -> [p, a*b]"""
    return ap.rearrange("p a b -> p (a b)")


# ------------------------------------------------------------ program build
def build_program():
    nc = bass.Bass("TRN2", target_bir_lowering=False, debug=False, num_devices=N_CORES)

    dt_in = {}
    for name, shape in [
        ("qT", [D, LQ]),
        ("kwT", [D, NTOK]),
        ("vwT", [D, NTOK]),
        ("ksT", [D, NB]),
        ("vsT", [D, NB]),
        ("Wqs", [D, H * DK]),
        ("Wks", [D, H * DK]),
        ("Wvs", [D, H * DV]),
        ("Wqw", [D, H * DK]),
        ("Wkw", [D, H * DK]),
        ("Wvw", [D, H * DV]),
        ("Wfc", [D + H * DV, D]),
        ("Wfc1", [H * DV, D]),
    ]:
        dt_in[name] = nc.dram_tensor(name, shape, FR, kind="ExternalInput").ap()
    for name, shape in [
        ("bqsT", [128, 4]),
        ("bksT", [128, 4]),
        ("bqwT", [128, 4]),
        ("bkwT", [128, 4]),
        ("bvsT", [128, 4]),
        ("bfc1T", [128, 4]),
        ("bfcT", [128, 4]),
        ("bvw", [H * DV]),
    ]:
        dt_in[name] = nc.dram_tensor(name, shape, FP, kind="ExternalInput").ap()
    outT_d = nc.dram_tensor("outT", [D, LQ], FP, kind="ExternalOutput").ap()

    with tile.TileContext(nc) as tc:
        # ------------------------------------------------ persistent pools
        ppool_cm = tc.tile_pool(name="persist", bufs=1)
        ppool = ppool_cm.__enter__()
        scpool_cm = tc.tile_pool(name="scps", bufs=2, space="PSUM")
        scpool = scpool_cm.__enter__()
        s3pool_cm = tc.tile_pool(name="s3ps", bufs=2, space="PSUM")
        s3pool = s3pool_cm.__enter__()
        ewpool_cm = tc.tile_pool(name="ewp", bufs=8)
        ewpool = ewpool_cm.__enter__()
        smpool_cm = tc.tile_pool(name="small", bufs=4)
        smpool = smpool_cm.__enter__()

        ident = ppool.tile([128, 128], FP, tag="ident")

        # biases (per-partition [128,4] layouts prepared on host)
        btiles = {}
        for bn in ["bqsT", "bksT", "bqwT", "bkwT", "bvsT", "bfc1T", "bfcT"]:
            t = ppool.tile([128, 4], FP, tag=bn)
            nc.sync.dma_start(out=t[:], in_=dt_in[bn][:])
            btiles[bn] = t
        bvw_bc = ppool.tile([128, H * DV], FP, tag="bvw_bc")
        src = dt_in["bvw"]
        bcast_ap = bass.AP(src.tensor, src.offset, [[0, 128]] + [list(x) for x in src.ap])
        nc.sync.dma_start(out=bvw_bc[:], in_=bcast_ap)

        # persistent sbuf tensors
        qw_pad = ppool.tile([128, H, LQ], BF, tag="qw_pad")
        ks_sb = ppool.tile([128, 4, NB], FR, tag="ks_sb")
        attn_sb = ppool.tile([128, 4, H, NB], FP, tag="attn_sb")
        fc1T_sb = ppool.tile([128, 4, LQ], FR, tag="fc1T")
        kw_sb = ppool.tile([128, 4, NTOK], BF, tag="kw_sb")
        vw_sb = ppool.tile([128, NTC, H, DV + 1], BF, tag="vw_sb")
        ctx_acc = ppool.tile([128, 4, H * DV], FP, tag="ctx_acc")
        vs_sb = ppool.tile([NB, H * DV], FR, tag="vs_sb")
        ctx_sT = ppool.tile([128, 4, LQ], FR, tag="ctx_sT")


        # ------------------------------------------- phase 1+2: small branch
        with tc.tile_pool(name="phaseA", bufs=1) as apool, tc.tile_pool(
            name="wring", bufs=2
        ) as wpool:
            qT_sb = apool.tile([128, 4, LQ], FR, tag="qT_sb")
            ksT_sb = apool.tile([128, 4, NB], FR, tag="ksT_sb")
            vsT_sb = apool.tile([128, 4, NB], FR, tag="vsT_sb")
            qs_sb = apool.tile([128, 4, LQ], FR, tag="qs_sb")

            def load_W(wn, wdt=FR):
                t = wpool.tile([128, 4, D], wdt, tag="Wring", name=wn)
                for k in range(4):
                    nc.sync.dma_start(
                        out=t[:, k, :], in_=dt_in[wn][k * 128 : (k + 1) * 128, :]
                    )
                return t

            for k in range(4):
                nc.sync.dma_start(out=qT_sb[:, k, :], in_=dt_in["qT"][k * 128 : (k + 1) * 128, :])
                nc.sync.dma_start(out=ksT_sb[:, k, :], in_=dt_in["ksT"][k * 128 : (k + 1) * 128, :])
                nc.sync.dma_start(out=vsT_sb[:, k, :], in_=dt_in["vsT"][k * 128 : (k + 1) * 128, :])
            make_identity(nc, ident)

            # qw projection into zero-padded per-head layout: head h=2mo on
            # partitions 0:64, h=2mo+1 on 64:128 (rest stays zero so K=128
            # score matmuls against the full kw partition range are exact).
            nc.vector.memset(qw_pad[:, :, :], 0.0)
            Wqw_t = load_W("Wqw")
            for mo in range(4):
                ps = scpool.tile([128, 512], FP, tag="sc", name="qwps")
                for k in range(4):
                    nc.tensor.matmul(
                        ps[:],
                        Wqw_t[:, k, mo * 128 : (mo + 1) * 128],
                        qT_sb[:, k, :],
                        start=(k == 0),
                        stop=(k == 3),
                    )
                nc.vector.tensor_scalar_add(
                    qw_pad[0:64, 2 * mo, :], ps[0:64, :], btiles["bqwT"][0:64, mo : mo + 1]
                )
                nc.vector.tensor_scalar_add(
                    qw_pad[64:128, 2 * mo + 1, :],
                    ps[64:128, :],
                    btiles["bqwT"][64:128, mo : mo + 1],
                )
            # qs projection
            Wqs_t = load_W("Wqs")
            for mo in range(4):
                ps = scpool.tile([128, 512], FP, tag="sc", name="qsps")
                for k in range(4):
                    nc.tensor.matmul(
                        ps[:],
                        Wqs_t[:, k, mo * 128 : (mo + 1) * 128],
                        qT_sb[:, k, :],
                        start=(k == 0),
                        stop=(k == 3),
                    )
                nc.vector.tensor_scalar_add(
                    qs_sb[:, mo, :], ps[:], btiles["bqsT"][:, mo : mo + 1]
                )

            # ks projection: out [hdk(mo), nb]
            Wks_t = load_W("Wks")
            for mo in range(4):
                ps = scpool.tile([128, 512], FP, tag="sc", name="ksps")
                for k in range(4):
                    nc.tensor.matmul(
                        ps[:, 0:NB],
                        Wks_t[:, k, mo * 128 : (mo + 1) * 128],
                        ksT_sb[:, k, :],
                        start=(k == 0),
                        stop=(k == 3),
                    )
                nc.vector.tensor_scalar_add(
                    ks_sb[:, mo, :], ps[:, 0:NB], btiles["bksT"][:, mo : mo + 1]
                )

            # vs projection: out [nb, hdv]  (lhsT = vsT chunk, rhs = Wvs)
            Wvs_t = load_W("Wvs")
            ps = scpool.tile([128, 512], FP, tag="sc", name="vsps")
            for k in range(4):
                nc.tensor.matmul(
                    ps[0:NB, :],
                    vsT_sb[:, k, :],
                    Wvs_t[:, k, :],
                    start=(k == 0),
                    stop=(k == 3),
                )
            nc.scalar.activation(vs_sb[:, :], ps[0:NB, :], ACTF.Copy)

            # sentence attention, batched: (1) all score matmuls + exp,
            # (2) batched softmax on DVE, (3) transposes + ctx_sT
            ews_all = apool.tile([128, 4, H, NB], FP, tag="ews_all")
            for h in range(H):
                hp, po = h // 2, (h % 2) * 64
                ps = scpool.tile([128, 512], FP, tag="sc", name="sattps")
                for qo in range(4):
                    nc.tensor.matmul(
                        ps[:, qo * NB : (qo + 1) * NB],
                        qs_sb[po : po + 64, hp, qo * 128 : (qo + 1) * 128],
                        ks_sb[po : po + 64, hp, :],
                        start=True,
                        stop=True,
                    )
                nc.scalar.activation(
                    ews_all[:, :, h, :],
                    ps[:, 0 : 4 * NB].rearrange("p (a x) -> p a x", x=NB),
                    ACTF.Exp,
                    scale=SCALE,
                )
            for h in range(H):
                for qo in range(4):
                    den = smpool.tile([128, 1], FP, tag="den")
                    nc.vector.tensor_reduce(den[:, 0:1], ews_all[:, qo, h, :], AX, ALU.add)
                    rec = smpool.tile([128, 1], FP, tag="rec")
                    nc.vector.reciprocal(rec[:], den[:])
                    nc.vector.tensor_scalar_mul(
                        attn_sb[:, qo, h, :], ews_all[:, qo, h, :], rec[:, 0:1]
                    )

        # ---------------- phase 3+4+5: projections interleaved with attention
        stpool_cm = tc.tile_pool(name="stage", bufs=2)
        stpool = stpool_cm.__enter__()

        Wkw_sb = ppool.tile([128, 4, H * DK], FR, tag="Wkw")
        Wvw_sb = ppool.tile([128, 4, H * DV], FR, tag="Wvw")
        for k in range(4):
            nc.sync.dma_start(out=Wkw_sb[:, k, :], in_=dt_in["Wkw"][k * 128 : (k + 1) * 128, :])
            nc.sync.dma_start(out=Wvw_sb[:, k, :], in_=dt_in["Wvw"][k * 128 : (k + 1) * 128, :])
        ctx_wT = ppool.tile([128, 4, LQ], FR, tag="ctx_wT")

        # kw/vw projection stages, interleaved into head 0's block loop
        def kw_stage(sg):
            stg = stpool.tile([128, 4, 1024], FR, tag="stg", name="kwstg")
            for k in range(4):
                nc.sync.dma_start(
                    out=stg[:, k, :],
                    in_=dt_in["kwT"][k * 128 : (k + 1) * 128, sg * 1024 : (sg + 1) * 1024],
                )
            for mo in range(4):
                ps = scpool.tile([128, 2, 512], FP, tag="sc", name="kwps")
                for j in range(2):
                    for k in range(4):
                        nc.tensor.matmul(
                            ps[:, j, :],
                            Wkw_sb[:, k, mo * 128 : (mo + 1) * 128],
                            stg[:, k, j * 512 : (j + 1) * 512],
                            start=(k == 0),
                            stop=(k == 3),
                        )
                nc.vector.tensor_scalar_add(
                    kw_sb[:, mo, sg * 1024 : (sg + 1) * 1024],
                    _flat2(ps),
                    btiles["bkwT"][:, mo : mo + 1],
                )

        def vw_stage(sg):
            stg = stpool.tile([128, 4, 1024], FR, tag="stg", name="vwstg")
            for k in range(4):
                nc.sync.dma_start(
                    out=stg[:, k, :],
                    in_=dt_in["vwT"][k * 128 : (k + 1) * 128, sg * 1024 : (sg + 1) * 1024],
                )
            for tp in range(4):
                ps = scpool.tile([128, 2, 512], FP, tag="sc", name="vwps")
                for j in range(2):
                    tcl = tp * 2 + j
                    for k in range(4):
                        nc.tensor.matmul(
                            ps[:, j, :],
                            stg[:, k, tcl * 128 : (tcl + 1) * 128],
                            Wvw_sb[:, k, :],
                            start=(k == 0),
                            stop=(k == 3),
                        )
                tc0 = sg * 8 + tp * 2
                nc.scalar.activation(
                    vw_sb[:, tc0 : tc0 + 2, :, 0:DV],
                    ps.rearrange("p a (h x) -> p a h x", x=DV),
                    ACTF.Copy,
                )
            nc.vector.memset(vw_sb[:, sg * 8 : (sg + 1) * 8, :, DV : DV + 1], 1.0)

        # token scores + ctx_w accumulation, software-pipelined: scores/exp
        # of pair p+1 are emitted before S3/STT of pair p so the in-order PE
        # never blocks on the exp of the pair it is about to consume.
        NP = NBH // 2  # pairs per head
        ew_store = {}

        def emit_scores(idx):
            h, npair = divmod(idx, NP)
            hp = h // 2
            if h == 0 and npair % 2 == 0:
                kw_stage(npair // 2)
                vw_stage(npair // 2)
            tiles = []
            for g in range(2):
                n = npair * 2 + g
                ps_sc = scpool.tile([128, 2, 512], FP, tag="sc", name="scps")
                for j in range(2):
                    tcg = 2 * n + j
                    nc.tensor.matmul(
                        ps_sc[:, j, :],
                        kw_sb[:, hp, tcg * 128 : (tcg + 1) * 128],
                        qw_pad[:, h, :],
                        start=True,
                        stop=True,
                    )
                ew_t = ewpool.tile([128, 2, 512], BF, tag="ew", name="ew_t")
                nc.scalar.activation(_flat2(ew_t), _flat2(ps_sc), ACTF.Exp, scale=SCALE)
                tiles.append(ew_t)
            ew_store[idx] = tiles

        def emit_s3(idx):
            h, npair = divmod(idx, NP)
            tiles = ew_store.pop(idx)
            s3t = s3pool.tile([128, 2, 512], FP, tag="s3", name="s3t")
            s3v = s3t.rearrange("p g (qo x) -> p g qo x", x=128)
            for g in range(2):
                n = npair * 2 + g
                for qo in range(4):
                    for j in range(2):
                        tcg = 2 * n + j
                        nc.tensor.matmul(
                            s3v[:, g, qo, 0 : DV + 1],
                            tiles[g][:, j, qo * 128 : (qo + 1) * 128],
                            vw_sb[:, tcg, h, :],
                            start=(j == 0),
                            stop=(j == 1),
                        )
            # factor = attn_s / denom for the 2x4 tiles in this super
            rec_t = smpool.tile([128, 2, 4], FP, tag="rec_t")
            nc.vector.reciprocal(rec_t[:], s3v[:, :, :, DV])
            fac_t = smpool.tile([128, 2, 4], FP, tag="fac_t")
            nc.vector.tensor_mul(
                fac_t[:],
                rec_t[:],
                attn_sb[:, :, h, npair * 2 : npair * 2 + 2].rearrange("p qo g -> p g qo"),
            )
            for g in range(2):
                n = npair * 2 + g
                for qo in range(4):
                    acc_sl = ctx_acc[:, qo, h * 64 : (h + 1) * 64]
                    if n == 0:
                        nc.vector.tensor_scalar(
                            acc_sl,
                            s3v[:, g, qo, 0:DV],
                            fac_t[:, g, qo : qo + 1],
                            None,
                            op0=ALU.mult,
                        )
                    else:
                        nc.vector.scalar_tensor_tensor(
                            acc_sl,
                            s3v[:, g, qo, 0:DV],
                            fac_t[:, g, qo : qo + 1],
                            acc_sl,
                            op0=ALU.mult,
                            op1=ALU.add,
                        )
            if npair == NP - 1:
                # head h complete: bias term + (for odd heads) ctx_w transpose
                for qo in range(4):
                    sh = smpool.tile([128, 1], FP, tag="sh")
                    nc.vector.tensor_reduce(
                        sh[:, 0:1], attn_sb[:, qo, h, 0:NBH], AX, ALU.add
                    )
                    nc.vector.scalar_tensor_tensor(
                        ctx_acc[:, qo, h * 64 : (h + 1) * 64],
                        bvw_bc[:, h * 64 : (h + 1) * 64],
                        sh[:, 0:1],
                        ctx_acc[:, qo, h * 64 : (h + 1) * 64],
                        op0=ALU.mult,
                        op1=ALU.add,
                    )
                if h % 2 == 1:
                    dc = h // 2
                    for qo in range(4):
                        ps = scpool.tile([128, 512], FP, tag="sc", name="ctps")
                        nc.tensor.transpose(
                            ps[:, 0:128],
                            ctx_acc[:, qo, dc * 128 : (dc + 1) * 128],
                            ident[:],
                        )
                        nc.scalar.activation(
                            ctx_wT[:, dc, qo * 128 : (qo + 1) * 128],
                            ps[:, 0:128],
                            ACTF.Copy,
                        )

        for idx in range(H * NP + 1):
            if idx < H * NP:
                emit_scores(idx)
            if idx >= 1:
                emit_s3(idx - 1)

        stpool_cm.__exit__(None, None, None)

        # ---------------- phase 6: sentence-ctx tail, then final fc
        lpool_cm = tc.tile_pool(name="late", bufs=1)
        lpool = lpool_cm.__enter__()
        aspool_cm = tc.tile_pool(name="asTring", bufs=2)
        aspool = aspool_cm.__enter__()
        outT_sb = lpool.tile([128, 4, LQ], FP, tag="outT_sb")
        Wfc_sb = lpool.tile([128, 8, D], FR, tag="Wfc")
        Wfc1_sb = lpool.tile([128, 4, D], FR, tag="Wfc1l")
        for k in range(8):
            nc.sync.dma_start(out=Wfc_sb[:, k, :], in_=dt_in["Wfc"][k * 128 : (k + 1) * 128, :])
        for k in range(4):
            nc.sync.dma_start(out=Wfc1_sb[:, k, :], in_=dt_in["Wfc1"][k * 128 : (k + 1) * 128, :])

        for h in range(H):
            hp, po = h // 2, (h % 2) * 64
            asT_h = aspool.tile([NB, 4, 128], FR, tag="asT", name="asT_h")
            for qo in range(4):
                psT = scpool.tile([128, 512], FP, tag="sc", name="sattT")
                nc.tensor.transpose(psT[0:NB, 0:128], attn_sb[:, qo, h, :], ident[:])
                nc.scalar.activation(asT_h[:, qo, :], psT[0:NB, 0:128], ACTF.Copy)
            # ctx_sT [dv(h), q] = vs.T @ attn_sT (+ b_vs per-partition)
            ps = scpool.tile([128, 512], FP, tag="sc", name="ctxsps")
            nc.tensor.matmul(
                ps[0:64, :],
                vs_sb[:, h * 64 : (h + 1) * 64],
                asT_h[:, :, :],
                start=True,
                stop=True,
            )
            nc.vector.tensor_scalar_add(
                ctx_sT[po : po + 64, hp, :],
                ps[0:64, :],
                btiles["bvsT"][po : po + 64, hp : hp + 1],
            )
        # fc1: out [dm(mo), q] = Wfc1.T @ ctx_sT, scaled 0.5 (+0.5*b_fc1)
        for mo in range(4):
            ps = scpool.tile([128, 512], FP, tag="sc", name="fc1ps")
            for k in range(4):
                nc.tensor.matmul(
                    ps[:],
                    Wfc1_sb[:, k, mo * 128 : (mo + 1) * 128],
                    ctx_sT[:, k, :],
                    start=(k == 0),
                    stop=(k == 3),
                )
            nc.vector.tensor_scalar(
                fc1T_sb[:, mo, :],
                ps[:],
                0.5,
                btiles["bfc1T"][:, mo : mo + 1],
                op0=ALU.mult,
                op1=ALU.add,
            )
        for mo in range(4):
            ps = scpool.tile([128, 512], FP, tag="sc", name="fcps")
            for cc in range(4):
                nc.tensor.matmul(
                    ps[:],
                    Wfc_sb[:, cc, mo * 128 : (mo + 1) * 128],
                    fc1T_sb[:, cc, :],
                    start=(cc == 0),
                    stop=False,
                )
            for cc in range(4):
                nc.tensor.matmul(
                    ps[:],
                    Wfc_sb[:, 4 + cc, mo * 128 : (mo + 1) * 128],
                    ctx_wT[:, cc, :],
                    start=False,
                    stop=(cc == 3),
                )
            nc.vector.tensor_scalar_add(
                outT_sb[:, mo, :], ps[:], btiles["bfcT"][:, mo : mo + 1]
            )
            nc.sync.dma_start(out=outT_d[mo * 128 : (mo + 1) * 128, :], in_=outT_sb[:, mo, :])

        aspool_cm.__exit__(None, None, None)
        lpool_cm.__exit__(None, None, None)
        smpool_cm.__exit__(None, None, None)
        ewpool_cm.__exit__(None, None, None)
        s3pool_cm.__exit__(None, None, None)
        scpool_cm.__exit__(None, None, None)
        ppool_cm.__exit__(None, None, None)

    ns = _split_multi_waits(nc)
    print(f"[kernel] split {ns} extra sem waits onto NOPs", file=sys.stderr)
    return nc


_NC_CACHE = None


def _get_nc():
    global _NC_CACHE
    if _NC_CACHE is None:
        _NC_CACHE = build_program()
    return _NC_CACHE


def make_in_maps(inputs):
    f = lambda x: np.ascontiguousarray(np.asarray(x, dtype=np.float32))
    q, k_w, v_w, k_s, v_s = (f(inputs[n]) for n in ["q", "k_w", "v_w", "k_s", "v_s"])
    W = {n: f(inputs[n]) for n in inputs if n.startswith(("W_", "b_"))}

    def bT(v, scale=1.0):
        return np.ascontiguousarray((v * scale).reshape(4, 128).T)

    shared = {
        "Wqs": W["W_qs"], "Wks": W["W_ks"], "Wvs": W["W_vs"],
        "Wqw": W["W_qw"], "Wkw": W["W_kw"], "Wvw": W["W_vw"],
        "Wfc1": W["W_fc1"], "Wfc": W["W_fc"],
        "bqsT": bT(W["b_qs"]), "bksT": bT(W["b_ks"]), "bqwT": bT(W["b_qw"]),
        "bkwT": bT(W["b_kw"]), "bvsT": bT(W["b_vs"]),
        "bfc1T": bT(W["b_fc1"], 0.5), "bfcT": bT(W["b_fc"], 0.5),
        "bvw": W["b_vw"],
    }
    in_maps = []
    for c in range(N_CORES):
        b, half = divmod(c, 2)
        blk = slice(half * NBH, half * NBH + NBH)
        ks_r = np.roll(k_s[b], -half * NBH, axis=0)
        vs_r = np.roll(v_s[b], -half * NBH, axis=0)
        m = dict(shared)
        m["qT"] = np.ascontiguousarray(q[b].T)
        m["kwT"] = np.ascontiguousarray(k_w[b, blk].reshape(NTOK, D).T)
        m["vwT"] = np.ascontiguousarray(v_w[b, blk].reshape(NTOK, D).T)
        m["ksT"] = np.ascontiguousarray(ks_r.T)
        m["vsT"] = np.ascontiguousarray(vs_r.T)
        in_maps.append(m)
    return in_maps


def run_cores(inputs, trace=False):
    nc = _get_nc()
    in_maps = make_in_maps(inputs)
    res = run_bass_kernel_spmd(nc, in_maps, list(range(N_CORES)), trace=trace)
    return res


def assemble(res):
    out = np.empty((B, LQ, D), dtype=np.float32)
    for b in range(B):
        out[b] = (res.results[2 * b]["outT"] + res.results[2 * b + 1]["outT"]).T
    return out


def kernel(**inputs) -> np.ndarray:
    res = run_cores(inputs, trace=False)
    return assemble(res)


if __name__ == "__main__":
    import reference

    inp = {k: np.asarray(v) for k, v in reference.setup_inputs().items()}
    out = kernel(**inp)
    exp = np.asarray(reference.reference(**inp))
    err = np.abs(out - exp).max() / np.abs(exp).max()
    print("max rel err:", err)
